# revision 17
# baseline (speedup 1.0000x reference)
"""CrossModalAttention Trainium2 kernel (v2).

Per-core (data-parallel over batch B=8 -> 8 NeuronCores):
  y_b = softmax((x_b Wq)(x_b Wk)^T * SCALE * (1 + mask_b)) (x_b Wv) @ Wo + bo

Design (transposed-softmax layout):
  - Host pre-transposes x -> xT and (1+mask) -> m1T (bf16), folds SCALE
    into Wq, and replicates bo across 8 rows. All device-side transposes
    and the DMA-transpose xbar are eliminated.
  - Scores are computed transposed, sT[j, i], two heads per PE pass via
    row tiling at partitions 0/64; Q/K in bf16 so the moving operand
    streams at 1 cycle/row.
  - Softmax runs without max-subtraction (|t| <= ~12, exp is safe in
    fp32): DVE multiplies sT(PSUM) by m1T in place, ACT exponentiates
    PSUM->bf16, and the AV matmul uses a ones-augmented bf16 V (65th
    column) so denominators fall out of the same accumulation (row 64).
  - Normalization: denominator rows staged to SBUF, one batched DVE
    reciprocal_approx_fast per head pair, a single selector matmul
    broadcasts both reciprocal rows across 128 partitions, DVE multiply
    writes outT straight from PSUM.
  - y = outT^T-contracted matmul against Wo with bias folded in via an
    8-partition (1/8 x bo8) outer product into the same accumulation.
"""

import numpy as np

B, N, D = 8, 2048, 512
H, DH = 8, 64
SCALE = DH ** -0.5

IC_N, IC = 4, 512      # i-chunks
JT_N, JT = 16, 128     # j-tiles
P_N = 4                # head pairs

_built = {}


def _cfg():
    import os
    return {
        # bf16s: mask stored as (mask-0.5) bf16, applied as (m~+1.5)*s via
        #        scalar_tensor_tensor (4x finer quantization than plain bf16)
        # bf16:  mask stored as (1+mask) bf16, plain tensor_tensor multiply
        # hilo:  exact hi/lo bf16 pair reassembled on GPSIMD
        "M1": os.environ.get("KB_M1", "bf16s"),
        "QKBF16": os.environ.get("KB_QKBF16", "1") == "1",
        "INPLACE": os.environ.get("KB_INPLACE", "1") == "1",
        "MTB": int(os.environ.get("KB_MTB", "32")),
        "SPB": int(os.environ.get("KB_SPB", "2")),
        "EGB": int(os.environ.get("KB_EGB", "3")),
    }


def _build():
    cfg = _cfg()
    import concourse.tile as tile
    from concourse import bacc, mybir

    F32 = mybir.dt.float32
    F32R = mybir.dt.float32r
    BF16 = mybir.dt.bfloat16
    Exp = mybir.ActivationFunctionType.Exp
    MULT = mybir.AluOpType.mult
    ADD = mybir.AluOpType.add

    QK_DT = BF16 if cfg["QKBF16"] else F32R

    nc = bacc.Bacc()
    xT_d = nc.declare_dram_parameter("xT", [D, N], F32R, isOutput=False)
    sel_d = nc.declare_dram_parameter("sel", [33, 128], F32, isOutput=False)
    if cfg["M1"] in ("bf16", "bf16s"):
        m1t_d = nc.declare_dram_parameter("m1t", [N, N], BF16, isOutput=False)
    else:
        m1th_d = nc.declare_dram_parameter("m1th", [N, N], BF16, isOutput=False)
        m1tl_d = nc.declare_dram_parameter("m1tl", [N, N], BF16, isOutput=False)
    wq_d = nc.declare_dram_parameter("wq", [D, D], F32R, isOutput=False)
    wk_d = nc.declare_dram_parameter("wk", [D, D], F32R, isOutput=False)
    wv_d = nc.declare_dram_parameter("wv", [D, D], F32R, isOutput=False)
    wo_d = nc.declare_dram_parameter("wo", [D, D], F32R, isOutput=False)
    bo8_d = nc.declare_dram_parameter("bo8", [8, D], F32R, isOutput=False)
    y_d = nc.declare_dram_parameter("y", [N, D], F32, isOutput=True)

    with nc.allow_low_precision(reason="bf16/f32r attention pipeline"), \
         tile.TileContext(nc) as tc:
        with tc.tile_pool(name="persist", bufs=1) as pp, \
             tc.tile_pool(name="ph2", bufs=1) as p2:
            # constants: selector comes from the host (engine writes may only
            # start at partitions 0/32/64/96, so it can't be memset-built)
            sel = pp.tile([33, 128], F32, tag="sel")
            nc.gpsimd.dma_start(out=sel, in_=sel_d[:])
            ones8_f = pp.tile([8, 128], F32, tag="ones8_f")
            nc.vector.memset(ones8_f, 1.0)
            ones8 = pp.tile([8, 128], F32R, tag="ones8")
            nc.vector.tensor_copy(ones8[:], ones8_f[:])
            rc2s = [pp.tile([33, 512], F32, tag=f"rc2_{i}", name=f"rc2_{i}")
                    for i in range(2)]
            for t in rc2s:
                nc.vector.memset(t, 1.0)

            qT = [pp.tile([128, N], QK_DT, tag=f"qT{t}", name=f"qT{t}") for t in range(4)]
            kT = [pp.tile([128, N], QK_DT, tag=f"kT{t}", name=f"kT{t}") for t in range(4)]
            v_sb = [pp.tile([128, H * 65], BF16, tag=f"v{t}", name=f"v{t}")
                    for t in range(JT_N)]
            outT = [pp.tile([128, N], F32R, tag=f"oT{t}", name=f"oT{t}")
                    for t in range(4)]

            # mask tiles for one ic: issued early so DMA overlaps phase 1
            def load_mts(ic):
                lst = []
                for jt in range(JT_N):
                    if cfg["M1"] in ("bf16", "bf16s"):
                        mt = p2.tile([128, IC], BF16, tag="mt", bufs=cfg["MTB"],
                                     name="mt")
                        nc.gpsimd.dma_start(
                            out=mt,
                            in_=m1t_d[jt * 128:(jt + 1) * 128, ic * IC:(ic + 1) * IC])
                    else:
                        mh = p2.tile([128, IC], BF16, tag="mth", bufs=8, name="mth")
                        ml = p2.tile([128, IC], BF16, tag="mtl", bufs=8, name="mtl")
                        nc.gpsimd.dma_start(
                            out=mh,
                            in_=m1th_d[jt * 128:(jt + 1) * 128, ic * IC:(ic + 1) * IC])
                        nc.sync.dma_start(
                            out=ml,
                            in_=m1tl_d[jt * 128:(jt + 1) * 128, ic * IC:(ic + 1) * IC])
                        mt = p2.tile([128, IC], F32, tag="mt", bufs=cfg["MTB"],
                                     name="mt")
                        nc.gpsimd.tensor_tensor(out=mt[:], in0=mh[:], in1=ml[:], op=ADD)
                    lst.append(mt)
                return lst

            mts_cur = load_mts(0)

            # ---------------- phase 1: projections ----------------
            with tc.tile_pool(name="ph01", bufs=1) as p1, \
                 tc.tile_pool(name="ph01ps", bufs=1, space="PSUM") as p1p:
                wq_sb = [p1.tile([128, D], F32R, tag=f"wq{c}", name=f"wq{c}") for c in range(4)]
                wk_sb = [p1.tile([128, D], F32R, tag=f"wk{c}", name=f"wk{c}") for c in range(4)]
                wv_sb = [p1.tile([128, D], F32R, tag=f"wv{c}", name=f"wv{c}") for c in range(4)]
                xT = [p1.tile([128, N], F32R, tag=f"xT{c}", name=f"xT{c}") for c in range(4)]
                for c in range(4):
                    nc.sync.dma_start(out=xT[c], in_=xT_d[c * 128:(c + 1) * 128, :])
                    nc.sync.dma_start(out=wq_sb[c], in_=wq_d[c * 128:(c + 1) * 128, :])
                    nc.sync.dma_start(out=wk_sb[c], in_=wk_d[c * 128:(c + 1) * 128, :])
                    nc.sync.dma_start(out=wv_sb[c], in_=wv_d[c * 128:(c + 1) * 128, :])

                for w_sb, dstT in ((wq_sb, qT), (wk_sb, kT)):
                    for hdt in range(4):
                        for nch in range(4):
                            qp = p1p.tile([128, 512], F32, tag="qp", bufs=2, name="qp")
                            for c in range(4):
                                nc.tensor.matmul(
                                    qp[:], w_sb[c][:, hdt * 128:(hdt + 1) * 128],
                                    xT[c][:, nch * 512:(nch + 1) * 512],
                                    start=(c == 0), stop=(c == 3))
                            nc.scalar.copy(dstT[hdt][:, nch * 512:(nch + 1) * 512], qp[:])

                for nt in range(JT_N):
                    vp = p1p.tile([128, 512], F32, tag="vp", bufs=2, name="vp")
                    for c in range(4):
                        nc.tensor.matmul(vp[:], xT[c][:, nt * 128:(nt + 1) * 128],
                                         wv_sb[c][:], start=(c == 0), stop=(c == 3))
                    nc.vector.memset(v_sb[nt], 1.0)
                    vdst = v_sb[nt].rearrange("p (h e) -> p h e", e=65)
                    nc.vector.tensor_copy(vdst[:, :, 0:64],
                                          vp[:].rearrange("p (h e) -> p h e", e=64))

            # ---------------- phase 2: attention + y emission ----------------
            wo_sb = [pp.tile([128, D], F32R, tag=f"wo{c}", name=f"wo{c}") for c in range(4)]
            for c in range(4):
                nc.gpsimd.dma_start(out=wo_sb[c], in_=wo_d[c * 128:(c + 1) * 128, :])
            bo8_sb = pp.tile([8, D], F32R, tag="bo8", name="bo8")
            nc.gpsimd.dma_start(out=bo8_sb, in_=bo8_d[:])

            with tc.tile_pool(name="ph2ps", bufs=1, space="PSUM") as p2p:
                for ic in range(IC_N):
                    mts_next = load_mts(ic + 1) if ic + 1 < IC_N else None
                    for p in range(P_N):
                        av0 = p2p.tile([65, 512], F32, tag="av0", name="av0")
                        av1 = p2p.tile([65, 512], F32, tag="av1", name="av1")
                        for jt in range(JT_N):
                            sp = p2p.tile([128, 1024], F32, tag="sp",
                                          bufs=cfg["SPB"], name="sp")
                            nc.tensor.matmul(
                                sp[:, 0:512],
                                kT[p][0:64, jt * 128:(jt + 1) * 128],
                                qT[p][0:64, ic * IC:(ic + 1) * IC],
                                start=True, stop=True, tile_position=(0, 0))
                            nc.tensor.matmul(
                                sp[:, 512:1024],
                                kT[p][64:128, jt * 128:(jt + 1) * 128],
                                qT[p][64:128, ic * IC:(ic + 1) * IC],
                                start=True, stop=True, tile_position=(64, 0))
                            spv = sp[:].rearrange("p (h i) -> p h i", h=2)
                            mbc = mts_cur[jt][:, None, :].broadcast_to((128, 2, IC))
                            if cfg["INPLACE"]:
                                tdst = spv
                                esrc = sp
                            else:
                                tg = p2.tile([128, 1024], F32, tag="tg", bufs=2,
                                             name="tg")
                                tdst = tg[:].rearrange("p (h i) -> p h i", h=2)
                                esrc = tg
                            if cfg["M1"] == "bf16s":
                                # t = (m~ + 1.5) * s
                                nc.vector.scalar_tensor_tensor(
                                    out=tdst, in0=mbc, scalar=1.5, in1=spv,
                                    op0=ADD, op1=MULT)
                            else:
                                nc.vector.tensor_tensor(
                                    out=tdst, in0=spv, in1=mbc, op=MULT)
                            e_g = p2.tile([128, 1024], BF16, tag="e_g",
                                          bufs=cfg["EGB"], name="e_g")
                            nc.scalar.activation(e_g[:], esrc[:], Exp)
                            for h in range(2):
                                hh = 2 * p + h
                                nc.tensor.matmul(
                                    (av0 if h == 0 else av1)[:],
                                    v_sb[jt][:, hh * 65:(hh + 1) * 65],
                                    e_g[:, h * 512:(h + 1) * 512],
                                    start=(jt == 0), stop=(jt == JT_N - 1))
                        # denominator rows staged to partitions 0/32 via ACT
                        # copies (partition-shifting standard ops only), then
                        # one aligned batched fast reciprocal; rows 1-31 hold
                        # the startup memset 1.0 so the zero-weighted selector
                        # rows contract cleanly
                        rc2 = rc2s[(ic * P_N + p) % 2]
                        nc.scalar.copy(rc2[0:1, :], av0[64:65, :])
                        nc.scalar.copy(rc2[32:33, :], av1[64:65, :])
                        rr2 = p2.tile([33, 512], F32, tag="rr2", bufs=2, name="rr2")
                        nc.vector.reciprocal_approx_fast(out=rr2[:], in_=rc2[:])
                        bc = p2p.tile([128, 512], F32, tag="bcy", bufs=2, name="bc")
                        nc.tensor.matmul(bc[:], sel[:], rr2[:], start=True, stop=True)
                        # norm reads av from PSUM; second operand must be SBUF
                        bcs = p2.tile([128, 512], F32, tag="bcs", bufs=2, name="bcs")
                        nc.scalar.copy(bcs[:], bc[:])
                        nc.vector.tensor_tensor(
                            out=outT[p][0:64, ic * IC:(ic + 1) * IC],
                            in0=av0[0:64, :], in1=bcs[0:64, :], op=MULT)
                        nc.vector.tensor_tensor(
                            out=outT[p][64:128, ic * IC:(ic + 1) * IC],
                            in0=av1[0:64, :], in1=bcs[64:128, :], op=MULT)

                    # y emission for this ic block
                    for itl in range(4):
                        it = ic * 4 + itl
                        yp = p2p.tile([128, 512], F32, tag="bcy", bufs=2, name="yp")
                        nc.tensor.matmul(yp[:], ones8[:], bo8_sb[:],
                                         start=True, stop=False)
                        for hdt in range(4):
                            nc.tensor.matmul(yp[:], outT[hdt][:, it * 128:(it + 1) * 128],
                                             wo_sb[hdt][:], start=False, stop=(hdt == 3))
                        y_sb = p2.tile([128, D], F32, tag="y_sb", bufs=2, name="y_sb")
                        nc.scalar.copy(y_sb[:], yp[:])
                        nc.gpsimd.dma_start(out=y_d[it * 128:(it + 1) * 128, :], in_=y_sb[:])
                    mts_cur = mts_next

    nc.finalize()
    return nc


def _get_nc():
    if "nc" not in _built:
        _built["nc"] = _build()
    return _built["nc"]


def _prep_in_maps(x, mask, Wq, Wk, Wv, Wo, bo):
    import ml_dtypes
    cfg = _cfg()
    x = np.asarray(x, dtype=np.float32)
    mask = np.asarray(mask, dtype=np.float32)
    xT = np.ascontiguousarray(x.transpose(0, 2, 1))
    m1t = np.ascontiguousarray((1.0 + mask).transpose(0, 2, 1))
    wq = np.asarray(Wq, dtype=np.float32) * SCALE
    wk = np.asarray(Wk, dtype=np.float32)
    wv = np.asarray(Wv, dtype=np.float32)
    wo = np.asarray(Wo, dtype=np.float32)
    # bias folded in as ones8.T @ bo8 with ones8 = 1 -> host pre-divides by 8
    bo8 = np.tile(np.asarray(bo, dtype=np.float32).reshape(1, D) / 8.0, (8, 1))
    sel = np.zeros((33, 128), dtype=np.float32)
    sel[0, 0:64] = 1.0
    sel[32, 64:128] = 1.0

    in_maps = []
    for b in range(B):
        m = {"xT": xT[b], "wq": wq, "wk": wk, "wv": wv, "wo": wo, "bo8": bo8,
             "sel": sel}
        if cfg["M1"] == "bf16s":
            m["m1t"] = (m1t[b] - 1.5).astype(ml_dtypes.bfloat16)
        elif cfg["M1"] == "bf16":
            m["m1t"] = m1t[b].astype(ml_dtypes.bfloat16)
        else:
            m1th = m1t[b].astype(ml_dtypes.bfloat16)
            m["m1th"] = m1th
            m["m1tl"] = (m1t[b] - m1th.astype(np.float32)).astype(ml_dtypes.bfloat16)
        in_maps.append(m)
    return in_maps


def kernel(x, mask, Wq, Wk, Wv, Wo, bo):
    from concourse.bass_utils import run_bass_kernel_spmd

    nc = _get_nc()
    in_maps = _prep_in_maps(x, mask, Wq, Wk, Wv, Wo, bo)
    res = run_bass_kernel_spmd(nc, in_maps, list(range(B)))
    return np.stack([res.results[b]["y"] for b in range(B)], axis=0)
